# revision 12
# baseline (speedup 1.0000x reference)
"""7x7 valid conv2d (cross-correlation) on a 4096x4096 fp32 image, 8 NeuronCores.

Strategy: shard output rows across 8 cores (512 rows each, overlapping input
slabs of 518 rows = halo baked in on the host, so no device collectives).
Per core the conv is computed on the TensorEngine as 7 PSUM-accumulated
"banded Toeplitz" matmuls: for each kernel column kx, a [K=128, M=122]
stationary matrix T_kx with T_kx[m+ky, m] = w[ky, kx] contracts 128 input
rows into 122 output rows; the kx shift is a free column offset on the
moving operand.  out[m, n] = sum_kx sum_ky w[ky,kx] * x[m+ky, n+kx].
"""

import numpy as np
import ml_dtypes

import concourse.bacc as bacc
import concourse.bass as bass
import concourse.tile as tile
import concourse.mybir as mybir
from concourse.bass_utils import run_bass_kernel_spmd

H = W = 4096
KH = KW = 7
OH = OW = H - KH + 1  # 4090
NCORES = 8
ROWS_PC = 512                 # output rows per core (core 7 re-computes 6 rows)
IN_ROWS = ROWS_PC + KH - 1    # 518 input rows per core
MT = 122                      # output rows per row-tile (contraction K = MT+6 = 128)
NT = 512                      # output cols per psum tile (one fp32 PSUM bank)
ROW_STARTS = list(range(0, ROWS_PC, MT))          # 0,122,244,366,488
COL_STARTS = list(range(0, OW, NT))               # 0,512,...,3584

MODE = "bf16"   # "bf16" | "fp32" | "f32r"
TRACE = False
LAST_EXEC_NS = None

_DT = {
    "bf16": (mybir.dt.bfloat16, ml_dtypes.bfloat16),
    "fp32": (mybir.dt.float32, np.float32),
    "f32r": (mybir.dt.float32r, np.float32),
}

_compiled = {}


def _build(mode):
    dt_b, _ = _DT[mode]
    nc = bacc.Bacc(
        "TRN2", target_bir_lowering=False, debug=False, num_devices=NCORES
    )
    x_d = nc.dram_tensor("x", [IN_ROWS, W], dt_b, kind="ExternalInput").ap()
    t_d = nc.dram_tensor("tmats", [128, KW * MT], dt_b, kind="ExternalInput").ap()
    o_d = nc.dram_tensor(
        "out", [ROWS_PC, OW], mybir.dt.float32, kind="ExternalOutput"
    ).ap()

    with tile.TileContext(nc) as tc:
        with (
            tc.tile_pool(name="tmat", bufs=1) as tpool,
            tc.tile_pool(name="xsl", bufs=3) as xpool,
            tc.tile_pool(name="ps", bufs=8, space="PSUM") as ppool,
            tc.tile_pool(name="ost", bufs=3) as opool,
        ):
            tm = tpool.tile([128, KW * MT], dt_b)
            nc.sync.dma_start(tm[:], t_d[:])
            # small 24-row tile first: its slab load is 0.25 MB, so the PE
            # starts ~5 us earlier while the big slabs stream in behind it
            row_order = [ROW_STARTS[-1]] + ROW_STARTS[:-1]
            for ti, r0 in enumerate(row_order):
                M = min(MT, ROWS_PC - r0)
                K = M + KH - 1
                xt = xpool.tile([128, W], dt_b, tag="x")
                if ti <= 1:
                    # split early slab loads so left-column matmuls start
                    # before the right half lands
                    nc.sync.dma_start(xt[:K, : W // 2], x_d[r0 : r0 + K, : W // 2])
                    nc.sync.dma_start(xt[:K, W // 2 :], x_d[r0 : r0 + K, W // 2 :])
                else:
                    nc.sync.dma_start(xt[:K, :], x_d[r0 : r0 + K, :])
                ot = opool.tile([128, OW], mybir.dt.float32, tag="o")
                for ci, c0 in enumerate(COL_STARTS):
                    N = min(NT, OW - c0)
                    ps = ppool.tile([MT, NT], mybir.dt.float32, tag="ps")
                    for kx in range(KW):
                        nc.tensor.matmul(
                            ps[:M, :N],
                            tm[:K, kx * MT : kx * MT + M],
                            xt[:K, c0 + kx : c0 + kx + N],
                            start=(kx == 0),
                            stop=(kx == KW - 1),
                        )
                    nc.vector.tensor_copy(ot[:M, c0 : c0 + N], ps[:M, :N])
                    # ship ~500 KB column blocks as soon as both col-tiles are
                    # copied. SWDGE (gpsimd) stores: HWDGE assigns a
                    # contiguous-DRAM store's descriptors to only 2 SDMA
                    # engines (~53 GB/s); SWDGE sprays wide.
                    if ci % 2 == 1:
                        lo = (ci - 1) * NT
                        hi = min((ci + 1) * NT, OW)
                        nc.gpsimd.dma_start(
                            o_d[r0 : r0 + M, lo:hi], ot[:M, lo:hi]
                        )
    nc.compile()
    return nc


def _toeplitz(weight, np_dt):
    t = np.zeros((128, KW * MT), dtype=np.float32)
    idx = np.arange(MT)
    for kx in range(KW):
        for ky in range(KH):
            t[idx + ky, kx * MT + idx] = weight[ky, kx]
    return np.ascontiguousarray(t.astype(np_dt))


def kernel(x, weight):
    global LAST_EXEC_NS
    mode = MODE
    dt_b, np_dt = _DT[mode]
    if mode not in _compiled:
        _compiled[mode] = _build(mode)
    nc = _compiled[mode]

    xf = np.asarray(x, np.float32)
    wf = np.asarray(weight, np.float32)
    tmats = _toeplitz(wf, np_dt)
    xc = xf.astype(np_dt) if np_dt is not np.float32 else xf

    starts = [min(i * ROWS_PC, OH - ROWS_PC) for i in range(NCORES)]
    in_maps = [
        {"x": np.ascontiguousarray(xc[s : s + IN_ROWS, :]), "tmats": tmats}
        for s in starts
    ]
    res = run_bass_kernel_spmd(
        nc, in_maps, core_ids=list(range(NCORES)), trace=TRACE
    )
    LAST_EXEC_NS = res.exec_time_ns

    out = np.empty((OH, OW), np.float32)
    for i, s in enumerate(starts):
        out[s : s + ROWS_PC, :] = res.results[i]["out"]
    return out


# revision 13
# speedup vs baseline: 1.2196x; 1.2196x over previous
"""7x7 valid conv2d (cross-correlation) on a 4096x4096 fp32 image, 8 NeuronCores.

Strategy: shard output rows across 8 cores (512 rows each, overlapping input
slabs of 518 rows = halo baked in on the host, so no device collectives).
Per core the conv is computed on the TensorEngine as 7 PSUM-accumulated
"banded Toeplitz" matmuls: for each kernel column kx, a [K=128, M=122]
stationary matrix T_kx with T_kx[m+ky, m] = w[ky, kx] contracts 128 input
rows into 122 output rows; the kx shift is a free column offset on the
moving operand.  out[m, n] = sum_kx sum_ky w[ky,kx] * x[m+ky, n+kx].
"""

import numpy as np
import ml_dtypes

import concourse.bacc as bacc
import concourse.bass as bass
import concourse.tile as tile
import concourse.mybir as mybir
from concourse.bass_utils import run_bass_kernel_spmd

H = W = 4096
KH = KW = 7
OH = OW = H - KH + 1  # 4090
NCORES = 8
ROWS_PC = 512                 # output rows per core (core 7 re-computes 6 rows)
IN_ROWS = ROWS_PC + KH - 1    # 518 input rows per core
MT = 122                      # output rows per row-tile (contraction K = MT+6 = 128)
NT = 512                      # output cols per psum tile (one fp32 PSUM bank)
ROW_STARTS = list(range(0, ROWS_PC, MT))          # 0,122,244,366,488
COL_STARTS = list(range(0, OW, NT))               # 0,512,...,3584

MODE = "bf16"   # "bf16" | "fp32" | "f32r"
TRACE = False
LAST_EXEC_NS = None

_DT = {
    "bf16": (mybir.dt.bfloat16, ml_dtypes.bfloat16),
    "fp32": (mybir.dt.float32, np.float32),
    "f32r": (mybir.dt.float32r, np.float32),
}

_compiled = {}


def _build(mode):
    dt_b, _ = _DT[mode]
    nc = bacc.Bacc(
        "TRN2", target_bir_lowering=False, debug=False, num_devices=NCORES
    )
    x_d = nc.dram_tensor("x", [IN_ROWS, W], dt_b, kind="ExternalInput").ap()
    t_d = nc.dram_tensor("tmats", [128, KW * MT], dt_b, kind="ExternalInput").ap()
    o_d = nc.dram_tensor(
        "out", [ROWS_PC, OW], mybir.dt.float32, kind="ExternalOutput"
    ).ap()

    with tile.TileContext(nc) as tc:
        with (
            tc.tile_pool(name="tmat", bufs=1) as tpool,
            tc.tile_pool(name="xsl", bufs=3) as xpool,
            tc.tile_pool(name="ps", bufs=8, space="PSUM") as ppool,
            tc.tile_pool(name="ost", bufs=3) as opool,
        ):
            tm = tpool.tile([128, KW * MT], dt_b)
            nc.sync.dma_start(tm[:], t_d[:])
            # big tiles first, 24-row tile last: the final stripe's store is
            # then only ~0.4 MB, keeping the post-matmul drain tail short
            row_order = list(ROW_STARTS)
            for ti, r0 in enumerate(row_order):
                M = min(MT, ROWS_PC - r0)
                K = M + KH - 1
                xt = xpool.tile([128, W], dt_b, tag="x")
                if ti <= 1:
                    # split early slab loads so left-column matmuls start
                    # before the right half lands
                    nc.sync.dma_start(xt[:K, : W // 2], x_d[r0 : r0 + K, : W // 2])
                    nc.sync.dma_start(xt[:K, W // 2 :], x_d[r0 : r0 + K, W // 2 :])
                else:
                    nc.sync.dma_start(xt[:K, :], x_d[r0 : r0 + K, :])
                ot = opool.tile([128, OW], mybir.dt.float32, tag="o")
                for ci, c0 in enumerate(COL_STARTS):
                    N = min(NT, OW - c0)
                    ps = ppool.tile([MT, NT], mybir.dt.float32, tag="ps")
                    for kx in range(KW):
                        nc.tensor.matmul(
                            ps[:M, :N],
                            tm[:K, kx * MT : kx * MT + M],
                            xt[:K, c0 + kx : c0 + kx + N],
                            start=(kx == 0),
                            stop=(kx == KW - 1),
                        )
                    nc.vector.tensor_copy(ot[:M, c0 : c0 + N], ps[:M, :N])
                    # ship ~500 KB column blocks as soon as both col-tiles are
                    # copied. SWDGE (gpsimd) stores: HWDGE assigns a
                    # contiguous-DRAM store's descriptors to only 2 SDMA
                    # engines (~53 GB/s); SWDGE sprays wide.
                    if ci % 2 == 1:
                        lo = (ci - 1) * NT
                        hi = min((ci + 1) * NT, OW)
                        nc.gpsimd.dma_start(
                            o_d[r0 : r0 + M, lo:hi], ot[:M, lo:hi]
                        )
    nc.compile()
    return nc


def _toeplitz(weight, np_dt):
    t = np.zeros((128, KW * MT), dtype=np.float32)
    idx = np.arange(MT)
    for kx in range(KW):
        for ky in range(KH):
            t[idx + ky, kx * MT + idx] = weight[ky, kx]
    return np.ascontiguousarray(t.astype(np_dt))


def kernel(x, weight):
    global LAST_EXEC_NS
    mode = MODE
    dt_b, np_dt = _DT[mode]
    if mode not in _compiled:
        _compiled[mode] = _build(mode)
    nc = _compiled[mode]

    xf = np.asarray(x, np.float32)
    wf = np.asarray(weight, np.float32)
    tmats = _toeplitz(wf, np_dt)
    xc = xf.astype(np_dt) if np_dt is not np.float32 else xf

    starts = [min(i * ROWS_PC, OH - ROWS_PC) for i in range(NCORES)]
    in_maps = [
        {"x": np.ascontiguousarray(xc[s : s + IN_ROWS, :]), "tmats": tmats}
        for s in starts
    ]
    res = run_bass_kernel_spmd(
        nc, in_maps, core_ids=list(range(NCORES)), trace=TRACE
    )
    LAST_EXEC_NS = res.exec_time_ns

    out = np.empty((OH, OW), np.float32)
    for i, s in enumerate(starts):
        out[s : s + ROWS_PC, :] = res.results[i]["out"]
    return out
